# revision 5
# baseline (speedup 1.0000x reference)
"""Trainium2 Bass kernel: single-head causal attention head layer.

Reference computation (per batch b):
    q = x[b] @ Wq; k = x[b] @ Wk; v = x[b] @ Wv        # [S, H], H=64
    w = softmax_causal(q @ k.T * E**-0.5)              # [S, S]
    out[b] = w @ v                                     # [S, H]

Shapes: x (8, 2048, 1024) f32, Wq/Wk/Wv (1024, 64) f32 -> out (8, 2048, 64) f32.

Sharding: data-parallel over batch, one batch per NeuronCore (8 cores).

Device algorithm (per core), all matmuls in bf16 with fp32 PSUM accumulation:
  1. qk^T projection: one pass over x^T with [Wq|Wk] stationary ->
     psum [128, 2048] (rows 0:64 = q^T, 64:128 = k^T).
  2. v projection: x^T tiles stationary, Wv moving -> v [s,64] per s-tile,
     stored as v_aug [128, 65] with a ones column (row sums come free later).
  3. Scores transposed: S^T[j,i] = k_j . q_i via lhsT=k^T[j] (64x128),
     rhs=q^T chunks. Keys on partitions -> softmax denominator is a
     partition-dim sum, folded into step 4.
  4. exp on ScalarE with scale=E**-0.5. No max-subtraction: scores are
     N(0, 0.0625) after scaling, |s|<~1.5, exp is safe in fp32.
     Causal mask = multiplicative 0/1 bf16 mask on diagonal chunks.
  5. O^T_aug[0:64,i] = sum_j v[j,h] P^T[j,i]; row 64 = column sums
     (softmax denominators), via the ones column of v_aug.
  6. PE-transpose O^T_aug 128-col slices -> [128, 65]; per-partition
     reciprocal of col 64; scale cols 0:64; DMA out fp32.
"""

import numpy as np
import ml_dtypes

BATCH = 8
SEQ = 2048
EMBED = 1024
HEAD = 64
N_CORES = 8
SCALE = float(EMBED) ** -0.5  # 0.03125

ST = SEQ // 128  # 16 seq tiles
ET = EMBED // 128  # 8 embed tiles
NCHUNK = SEQ // 512  # 4 chunks of 512 along the query dim

_CACHE = {}


def _build_program():
    import concourse.mybir as mybir
    from concourse import bacc
    from concourse.tile import TileContext

    f32 = mybir.dt.float32
    bf16 = mybir.dt.bfloat16
    EXP = mybir.ActivationFunctionType.Exp

    nc = bacc.Bacc("TRN2", target_bir_lowering=False, debug=False,
                   num_devices=N_CORES)

    xT = nc.declare_dram_parameter("xT", [EMBED, SEQ], bf16, isOutput=False)
    wqk = nc.declare_dram_parameter("wqk", [128, ET, 128], bf16, isOutput=False)
    wv = nc.declare_dram_parameter("wv", [128, ET, HEAD], bf16, isOutput=False)
    masks = nc.declare_dram_parameter("masks", [128, 4, 512], bf16,
                                      isOutput=False)
    ident = nc.declare_dram_parameter("ident", [HEAD + 1, HEAD + 1], f32,
                                      isOutput=False)
    out = nc.declare_dram_parameter("out", [SEQ, HEAD], f32, isOutput=True)

    with TileContext(nc) as tc:
        with (
            tc.tile_pool(name="persist", bufs=1) as persist,
            tc.tile_pool(name="vtiles", bufs=ST) as vtiles,
            tc.tile_pool(name="psb", bufs=3) as psb,
            tc.tile_pool(name="osb", bufs=3) as osb,
            tc.tile_pool(name="rsb", bufs=3) as rsb,
        ):
            # ---- Phase A: load inputs to SBUF ----
            xt_sb = persist.tile([128, ET, SEQ], bf16)
            for e in range(ET):
                nc.sync.dma_start(out=xt_sb[:, e, :],
                                  in_=xT[128 * e:128 * (e + 1), :])
            wqk_sb = persist.tile([128, ET, 128], bf16)
            nc.sync.dma_start(out=wqk_sb[:], in_=wqk[:])
            wv_sb = persist.tile([128, ET, HEAD], bf16)
            nc.sync.dma_start(out=wv_sb[:], in_=wv[:])
            mask_sb = persist.tile([128, 4, 512], bf16)
            nc.sync.dma_start(out=mask_sb[:], in_=masks[:])
            id_sb = persist.tile([HEAD + 1, HEAD + 1], f32)
            nc.sync.dma_start(out=id_sb[:], in_=ident[:])

            qk_sb = persist.tile([128, SEQ], bf16)  # rows 0:64 qT, 64:128 kT
            kt_sb = persist.tile([64, SEQ], bf16)  # kT moved to base partition 0
            v_sbs = []
            for s in range(ST):
                v_sbs.append(vtiles.tile([128, HEAD + 1], bf16,
                                         name=f"v{s}", tag=f"v{s}"))
            ot_sb = persist.tile([HEAD + 1, SEQ], f32)

            # ---- Phase B: q^T/k^T projection ----
            with tc.tile_pool(name="ps_b", bufs=1, space="PSUM") as ps_b, \
                 tc.tile_pool(name="ps_v", bufs=2, space="PSUM") as ps_v:
                qk_ps = ps_b.tile([128, SEQ], f32)
                for e in range(ET):
                    for c in range(NCHUNK):
                        nc.tensor.matmul(
                            qk_ps[:, 512 * c:512 * (c + 1)],
                            lhsT=wqk_sb[:, e, :],
                            rhs=xt_sb[:, e, 512 * c:512 * (c + 1)],
                            start=(e == 0), stop=(e == ET - 1),
                        )
                for c in range(NCHUNK):
                    nc.vector.tensor_copy(qk_sb[:, 512 * c:512 * (c + 1)],
                                          qk_ps[:, 512 * c:512 * (c + 1)])
                # PE matmul needs lhsT/rhs on the same base partition:
                # shift kT (partitions 64:128) down to a base-0 tile via DMA.
                nc.sync.dma_start(out=kt_sb[:], in_=qk_sb[64:128, :])

                # ---- Phase C: v projection (x^T tiles stationary) ----
                for s in range(ST):
                    v_ps = ps_v.tile([128, HEAD], f32, tag="vps")
                    for e in range(ET):
                        nc.tensor.matmul(
                            v_ps[:],
                            lhsT=xt_sb[:, e, 128 * s:128 * (s + 1)],
                            rhs=wv_sb[:, e, :],
                            start=(e == 0), stop=(e == ET - 1),
                        )
                    nc.vector.memset(v_sbs[s][:, HEAD:HEAD + 1], 1.0)
                    nc.vector.tensor_copy(v_sbs[s][:, 0:HEAD], v_ps[:])

            # ---- Phases D+E: attention ----
            with tc.tile_pool(name="ps_o", bufs=1, space="PSUM") as ps_o:
                o_ps = ps_o.tile([HEAD + 1, SEQ], f32)

                with tc.tile_pool(name="ps_s", bufs=3, space="PSUM") as ps_s:
                    for j in range(ST):
                        kT = kt_sb[:, 128 * j:128 * (j + 1)]
                        c0 = j // 4
                        m = j % 4
                        for c in range(c0, NCHUNK):
                            lo = 128 * m if c == c0 else 0
                            s_ps = ps_s.tile([128, 512], f32, tag="sps")
                            nc.tensor.matmul(
                                s_ps[:, lo:512],
                                lhsT=kT,
                                rhs=qk_sb[0:64, 512 * c + lo:512 * (c + 1)],
                                start=True, stop=True,
                            )
                            p_sb = psb.tile([128, 512], bf16, tag="psb")
                            if lo > 0:
                                nc.vector.memset(p_sb[:, 0:lo], 0.0)
                            nc.scalar.activation(p_sb[:, lo:512],
                                                 s_ps[:, lo:512],
                                                 EXP, scale=SCALE)
                            if c == c0:
                                nc.vector.tensor_mul(
                                    p_sb[:, lo:512], p_sb[:, lo:512],
                                    mask_sb[:, m, lo:512])
                            nc.tensor.matmul(
                                o_ps[:, 512 * c:512 * (c + 1)],
                                lhsT=v_sbs[j][:],
                                rhs=p_sb[:],
                                start=(j == 0), stop=(j == 4 * c + 3),
                            )

                    for c in range(NCHUNK):
                        nc.vector.tensor_copy(ot_sb[:, 512 * c:512 * (c + 1)],
                                              o_ps[:, 512 * c:512 * (c + 1)])

                # ---- Phase E: transpose + normalize + store ----
                with tc.tile_pool(name="ps_t", bufs=2, space="PSUM") as ps_t:
                    for s in range(ST):
                        t_ps = ps_t.tile([128, HEAD + 1], f32, tag="tps")
                        nc.tensor.transpose(
                            t_ps[:], ot_sb[:, 128 * s:128 * (s + 1)], id_sb[:])
                        recip = rsb.tile([128, 1], f32, tag="recip")
                        nc.vector.reciprocal(recip[:],
                                             t_ps[:, HEAD:HEAD + 1])
                        o_sb = osb.tile([128, HEAD], f32, tag="osb")
                        nc.vector.tensor_scalar_mul(o_sb[:], t_ps[:, 0:HEAD],
                                                    recip[:])
                        nc.sync.dma_start(
                            out=out[128 * s:128 * (s + 1), :], in_=o_sb[:])

    nc.compile()
    return nc


def _get_program():
    if "nc" not in _CACHE:
        _CACHE["nc"] = _build_program()
    return _CACHE["nc"]


def _host_inputs(x, Wq, Wk, Wv):
    bf16 = ml_dtypes.bfloat16
    # x^T per batch: [E, S] contiguous, bf16
    xT = np.ascontiguousarray(x.transpose(0, 2, 1)).astype(bf16)
    # [Wq | Wk] -> [128, ET, 128] (partition = embed % 128)
    wqk = np.concatenate([Wq, Wk], axis=1).astype(bf16)  # [E, 128]
    wqk = np.ascontiguousarray(
        wqk.reshape(ET, 128, 128).transpose(1, 0, 2))  # [128, ET, 128]
    wv = np.ascontiguousarray(
        Wv.astype(bf16).reshape(ET, 128, HEAD).transpose(1, 0, 2))
    # masks[m][x, y] = 1.0 iff y - x - 128*m >= 0
    xx = np.arange(128)[:, None]
    yy = np.arange(512)[None, :]
    masks = np.stack([(yy - xx - 128 * m >= 0) for m in range(4)],
                     axis=1).astype(bf16)  # [128, 4, 512]
    ident = np.eye(HEAD + 1, dtype=np.float32)
    return xT, wqk, wv, masks, ident


def kernel(x, Wq, Wk, Wv):
    from concourse.bass_utils import run_bass_kernel_spmd

    nc = _get_program()
    xT, wqk, wv, masks, ident = _host_inputs(x, Wq, Wk, Wv)
    in_maps = [
        {"xT": xT[b], "wqk": wqk, "wv": wv, "masks": masks, "ident": ident}
        for b in range(BATCH)
    ]
    res = run_bass_kernel_spmd(nc, in_maps, list(range(N_CORES)))
    out = np.stack([np.asarray(res.results[b]["out"]) for b in range(BATCH)])
    return out.astype(np.float32)
